# revision 1
# baseline (speedup 1.0000x reference)
"""CAAN kernel for Trainium2, 8-core data-parallel (one batch row per core).

Math: the reference is
    Q = R Wq^T + bq ; K = R Wk^T + bk ; V = R Wv^T + bv
    E = exp(Q K^T / sqrt(512)) ; saat = E / rowsum(E)
    winner = (saat V) W1^T W2^T + (W2 b1 + b2)

Two algebraic collapses make most of the network disappear:

1. The W1/W2 head is linear, so with c = W1^T W2[0]:
       winner[n] = (sum_m E[n,m] u[m]) / (sum_m E[n,m]) + const,
   u = V c = R (Wv^T c) + bv.c — a per-asset scalar. The V projection and
   attention*V matmul vanish.

2. gamma = Q K^T = R A R^T + (R Wq^T bk)[n] + (R Wk^T bq)[m] + bq.bk with
   A = Wq^T Wk. The per-n term scales E rows uniformly and cancels in the
   s/rowsum ratio, so it is dropped. The per-m term v[m] rides the exp
   activation's per-partition bias slot. The Q and K projections collapse
   into a single projection B = A^T-pack @ R^T.

Per-core device work (batch row b):
  phase A: B[q,m] = sum_q' A[q,q'] R[m,q'] (bf16, qc-outer waves so matmuls
           start when the first R chunk lands); u/v rows as M=1 projections,
           transposed to [128,16] columns via K=1 matmuls against a ones
           scalar.
  phase B: per 128-row m-chunk: gamma^T = B^T-slice @ R^T (PSUM fp32),
           exp(scale*psum + v) -> ET bf16 (ACT), then [u_chunk|ones]^T @ ET
           accumulates s[n] (partition 0) and rowsum[n] (partition 32).
           The s/rowsum matmuls trail one m-chunk behind the score matmuls
           so the PE never waits on exp.
  out: s and rowsum copied to SBUF, DMA'd to DRAM [2, 2048] f32; the host
       does winner = s/rowsum + const.
"""

import math

import ml_dtypes
import numpy as np

import concourse.bass as bass
import concourse.mybir as mybir
import concourse.tile as tile
from concourse.bass_utils import run_bass_kernel_spmd
from concourse.vector_clock import ScopedClock


N_CORES = 8
NB, NN, DD = 8, 2048, 512  # batch, assets, feature dim
P = 128
NQ = DD // P   # q chunks (contraction)
NM = NN // P   # m chunks (key/asset rows)
S = 512        # matmul moving free dim / PSUM bank width
NS = NN // S   # n slices of 512
BF16 = mybir.dt.bfloat16
F32 = mybir.dt.float32
SCALE = 1.0 / math.sqrt(float(DD))
BF = ml_dtypes.bfloat16


class _TileContext(tile.TileContext):
    """Workaround for walrus rejecting >1 sem wait on the kernel-tail Drain
    ("Too many sync wait commands"): put each final wait on its own SP NoOp
    ahead of an unwaited Drain."""

    def _drain_and_barrier(self, tick_clock, wait_clock):
        nc = self.nc
        probe = nc.sync.nop(nofuse=True)
        wait_clock.add_sem_waits(
            probe.ins, ScopedClock({None: tick_clock.global_clock})
        )
        si = probe.ins.sync_info
        waits = list(si.on_wait) if si is not None else []
        if si is not None:
            si.on_wait = []
        # spread the final waits round-robin over all engines so they
        # resolve in parallel; the barrier then guarantees every wait has
        # been observed before the SP drain runs.
        engines = [nc.sync, nc.vector, nc.scalar, nc.tensor, nc.gpsimd]
        for i, w in enumerate(waits):
            n = engines[i % len(engines)].nop(nofuse=True)
            n.ins.sync_info = mybir.SyncInfo(on_wait=[w], on_update=[])
        nc.all_engine_barrier()
        nc.sync.drain()
        assert self.sems is not None
        popped = nc._tile_sem_poison_stack.pop()
        assert popped is self._sem_poison
        # clear_and_free_semaphores would range-clear every ALLOCATED sem id
        # (~200+), which walrus lowers to one op per id (~7us of tail).
        # Only ids that appear in the final instruction stream can be
        # non-zero, so hardware-clear just those; do the allocator
        # bookkeeping for the full set.
        allocated = list(self.sems.allocated().values())
        sem_nums = [
            s.num if hasattr(s, "num") else int(s) for s in allocated
        ]
        used = set()
        for fn in nc.m.functions:
            for blk in fn.blocks:
                for inst in blk.instructions:
                    si = inst.sync_info
                    if si is not None:
                        for w in si.on_wait:
                            used.add(w.id)
                        for u in si.on_update:
                            used.add(u.id)
        hw_nums = sorted(n for n in sem_nums if n in used)
        for sem_range in bass.compact_to_ranges(hw_nums):
            nc.gpsimd.dma_reset(sem_range)
            nc.gpsimd.sem_clear(sem_range)
        nc._state.prepend_free_semaphores(sem_nums)
        for poison_set in nc._tile_sem_poison_stack:
            poison_set.update(sem_nums)
        # the trailing all_engine_barrier is skipped: nothing after the
        # clear touches semaphores, and the runtime serializes executions


def _split_multi_waits(nc, maxw=1):
    """This walrus build rejects instructions carrying more than one sync
    wait ("Too many sync wait commands"). Move excess waits onto same-engine
    NoOps inserted just before the instruction: sem-ge waits are monotonic
    within the kernel, so waiting for them earlier on the same engine is
    equivalent. sem-eq waits stay on the original instruction."""
    for fn in nc.m.functions:
        for blk in fn.blocks:
            insts = blk.instructions
            if not any(
                i.sync_info is not None and len(i.sync_info.on_wait) > maxw
                for i in insts
            ):
                continue
            out = []
            for inst in insts:
                si = inst.sync_info
                if si is not None and len(si.on_wait) > maxw:
                    keep = [w for w in si.on_wait if "eq" in w.wait_mode]
                    movable = [w for w in si.on_wait if "eq" not in w.wait_mode]
                    while len(keep) < maxw and movable:
                        keep.append(movable.pop(0))
                    assert len(keep) <= maxw, (
                        f"{inst.name}: {len(keep)} non-splittable waits"
                    )
                    for w in movable:
                        nop = mybir.InstNoOp(
                            name=nc.get_next_instruction_name(), ins=[], outs=[]
                        )
                        nop.engine = inst.engine
                        nop.sync_info = mybir.SyncInfo(on_wait=[w], on_update=[])
                        out.append(nop)
                    si.on_wait = keep
                out.append(inst)
            blk.instructions = out


def _build():
    nc = bass.Bass("TRN2", target_bir_lowering=False, debug=False)

    rt = nc.dram_tensor("rt", (NQ, P, NN), BF16, kind="ExternalInput")
    amat = nc.dram_tensor("amat", (NQ, P, DD), BF16, kind="ExternalInput")
    wuv = nc.dram_tensor("wuv", (NQ, P, 33), BF16, kind="ExternalInput")
    betas = nc.dram_tensor("betas", (33, 2), F32, kind="ExternalInput")
    out = nc.dram_tensor("out", (2, NN), F32, kind="ExternalOutput")

    Ident = mybir.ActivationFunctionType.Identity
    Copy = mybir.ActivationFunctionType.Copy
    Exp = mybir.ActivationFunctionType.Exp

    with _TileContext(nc) as tc:
        with (
            tc.tile_pool(name="const", bufs=1) as cpool,
            tc.tile_pool(name="big", bufs=1) as big,
            tc.tile_pool(name="et", bufs=4) as et_pool,
            tc.tile_pool(name="dscratch", bufs=1, space="DRAM") as dpool,
        ):
            # rt0 + amat chunks lead on the HWDGE (sync) queue so the first
            # projection wave can start ASAP; rt2/rt3 stream on SWDGE.
            rt_sb = [cpool.tile([P, NN], BF16, name=f"rt{qc}") for qc in range(NQ)]
            a_sb = [cpool.tile([P, DD], BF16, name=f"a{qc}") for qc in range(NQ)]
            # a0 (tiny) first, then rt0 split across both queue types so the
            # first projection wave's critical inputs land earliest
            nc.sync.dma_start(a_sb[0][:], amat.ap()[0])
            nc.sync.dma_start(rt_sb[0][:, : NN // 2], rt.ap()[0][:, : NN // 2])
            nc.gpsimd.dma_start(rt_sb[0][:, NN // 2 :], rt.ap()[0][:, NN // 2 :])
            wuv_sb = cpool.tile([P, NQ, 33], BF16)
            nc.gpsimd.dma_start(wuv_sb[:], wuv.ap().rearrange("q p c -> p q c"))
            betas_sb = cpool.tile([33, 2], F32)
            nc.gpsimd.dma_start(betas_sb[:], betas.ap())
            nc.gpsimd.dma_start(rt_sb[2][:], rt.ap()[2])
            nc.sync.dma_start(rt_sb[1][:], rt.ap()[1])
            nc.sync.dma_start(a_sb[1][:], amat.ap()[1])
            nc.gpsimd.dma_start(rt_sb[3][:], rt.ap()[3])
            nc.sync.dma_start(a_sb[2][:], amat.ap()[2])
            nc.sync.dma_start(a_sb[3][:], amat.ap()[3])

            bt_sb = [big.tile([P, NN], BF16, name=f"bt{qc}") for qc in range(NQ)]
            uvrow_sb = big.tile([33, NN], BF16)
            vcol_sb = big.tile([P, NM], BF16)
            v_sb = big.tile([P, NM], F32)
            # su columns: 0 = u, 32 = ones (s lands on partition 0, rowsum
            # on partition 32 -- both legal base partitions), rest zero.
            su_sb = big.tile([P, NM, 33], BF16)
            nc.vector.memset(su_sb[:], 0.0)
            nc.vector.memset(su_sb[:, :, 32:33], 1.0)

            # One PSUM pool serves projection, u/v and gamma tiles (same
            # tag -> same 4 rotating slots). No pool release between phases
            # means deps are per-slot instead of whole-zone, so phase B's
            # first matmuls don't wait on the entire phase-A cast clock.
            # srs gets the other 4 banks, allocated first and only touched
            # after exp(0).
            psR = tc.alloc_tile_pool(name="psR", bufs=1, space="PSUM")
            psMain = tc.alloc_tile_pool(name="psMain", bufs=4, space="PSUM")
            if True:
                def b_wave(qo):
                    pts = [
                        psMain.tile([P, S], F32, tag="mm", name="mm")
                        for _ in range(NS)
                    ]
                    for qi in range(NQ):
                        for ns in range(NS):
                            nc.tensor.matmul(
                                pts[ns][:],
                                a_sb[qi][:, qo * P : (qo + 1) * P],
                                rt_sb[qi][:, ns * S : (ns + 1) * S],
                                start=(qi == 0),
                                stop=(qi == NQ - 1),
                            )
                    for ns in range(NS):
                        nc.vector.tensor_copy(
                            bt_sb[qo][:, ns * S : (ns + 1) * S],
                            pts[ns][:],
                        )

                def uv_rows():
                    # one M=33 pass computes both u (partition 0) and v
                    # (partition 32, pre-scaled) from the [wtl|w2tl] lhsT
                    for ns in range(NS):
                        pur = psMain.tile([P, S], F32, tag="mm", name="mm")[0:33, :]
                        for qc in range(NQ):
                            nc.tensor.matmul(
                                pur[:],
                                wuv_sb[:, qc, :],
                                rt_sb[qc][:, ns * S : (ns + 1) * S],
                                start=(qc == 0),
                                stop=(qc == NQ - 1),
                            )
                        nc.scalar.activation(
                            uvrow_sb[0:1, ns * S : (ns + 1) * S],
                            pur[0:1, :],
                            Ident,
                            bias=betas_sb[0:1, 0:1],
                            scale=1.0,
                        )
                        nc.scalar.activation(
                            uvrow_sb[32:33, ns * S : (ns + 1) * S],
                            pur[32:33, :],
                            Ident,
                            bias=betas_sb[32:33, 1:2],
                            scale=SCALE,
                        )

                b_wave(0)
                uv_rows()
                b_wave(1)
                b_wave(2)
                b_wave(3)

                # scatter rows [1, 2048] -> columns [128, 16] off the PE:
                # bounce through flat DRAM, where the partition-scatter read
                # pattern is expressible.
                uv_dram = dpool.tile([2, NN], BF16)
                nc.sync.dma_start(uv_dram[0:1, :], uvrow_sb[0:1, :])
                nc.sync.dma_start(uv_dram[1:2, :], uvrow_sb[32:33, :])
                with nc.allow_non_contiguous_dma(
                    reason="2048-elem partition scatter, one-off"
                ):
                    nc.sync.dma_start(
                        su_sb[:, :, 0],
                        uv_dram[0, :].rearrange("(m p) -> p m", p=P),
                    )
                    nc.sync.dma_start(
                        vcol_sb[:],
                        uv_dram[1, :].rearrange("(m p) -> p m", p=P),
                    )
                nc.vector.tensor_copy(v_sb[:], vcol_sb[:])

            # ---- phase B: scores, exp, s/rowsum accumulation ----
            if True:
                srs = [
                    psR.tile([33, S], F32, tag=f"srs{ns}", name=f"srs{ns}")
                    for ns in range(NS)
                ]
                ets = {}

                def gamma(mc):
                    et = et_pool.tile([P, NN], BF16, tag="et", name="et")
                    ets[mc] = et
                    # ns-outer, one PSUM tile in flight at a time: each bank
                    # frees right after its exp, so three slots pipeline
                    # (walrus emits LDWEIGHTS per matmul regardless of loop
                    # order -- ldw-opt is force-disabled -- so the extra
                    # weight reloads here cost nothing extra).
                    for ns in range(NS):
                        g = psMain.tile([P, S], F32, tag="mm", name="mm")
                        for qc in range(NQ):
                            nc.tensor.matmul(
                                g[:],
                                bt_sb[qc][:, mc * P : (mc + 1) * P],
                                rt_sb[qc][:, ns * S : (ns + 1) * S],
                                start=(qc == 0),
                                stop=(qc == NQ - 1),
                            )
                        nc.scalar.activation(
                            et[:, ns * S : (ns + 1) * S],
                            g[:],
                            Exp,
                            bias=v_sb[:, mc : mc + 1],
                            scale=SCALE,
                        )

                def srs_mms(mc):
                    et = ets.pop(mc)
                    for ns in range(NS):
                        nc.tensor.matmul(
                            srs[ns][:],
                            su_sb[:, mc, :],
                            et[:, ns * S : (ns + 1) * S],
                            start=(mc == 0),
                            stop=(mc == NM - 1),
                            skip_group_check=True,
                        )

                # s/rowsum matmuls trail one m-chunk behind the score
                # matmuls so the PE never stalls on the exp activations.
                gamma(0)
                for mc in range(1, NM):
                    gamma(mc)
                    srs_mms(mc - 1)
                srs_mms(NM - 1)

                # copy PSUM -> SBUF (rows 0..32), then DMA rows 0 and 32 out
                out_sb = big.tile([33, NN], F32)
                for ns in range(NS):
                    sl = slice(ns * S, (ns + 1) * S)
                    # alternate DVE/ACT so the four drain copies run on two
                    # engines in parallel
                    if ns % 2 == 0:
                        nc.vector.tensor_copy(out_sb[:, sl], srs[ns][:])
                    else:
                        nc.scalar.copy(out_sb[:, sl], srs[ns][:])
                nc.sync.dma_start(out.ap()[0:1, :], out_sb[0:1, :])
                nc.sync.dma_start(out.ap()[1:2, :], out_sb[32:33, :])
            psMain.release()
            psR.release()

    _split_multi_waits(nc)
    return nc


_NC = None


def _get_nc():
    global _NC
    if _NC is None:
        _NC = _build()
    return _NC


def _pack_pq(a):
    """[512, X] -> [128, 4, X] with (p, chunk) partition striping."""
    return np.ascontiguousarray(a.reshape(4, P, -1).transpose(1, 0, 2))


def kernel(R, Wq, bq, Wk, bk, Wv, bv, W1, b1, W2, b2):
    R = np.asarray(R, np.float32)
    Wq = np.asarray(Wq, np.float64)
    bq = np.asarray(bq, np.float64)
    Wk = np.asarray(Wk, np.float64)
    bk = np.asarray(bk, np.float64)
    Wv = np.asarray(Wv, np.float64)
    bv = np.asarray(bv, np.float64)
    W1 = np.asarray(W1, np.float64)
    b1 = np.asarray(b1, np.float64)
    W2 = np.asarray(W2, np.float64)
    b2 = np.asarray(b2, np.float64)

    # Collapse the linear head: winner = c.a + const, u = V c.
    c = W1.T @ W2[0]                      # [512]
    wtilde = Wv.T @ c                     # [512]
    beta = float(bv @ c)
    const = float(W2[0] @ b1 + b2[0])
    # Collapse the Q/K projections: gamma = R A R^T + v[m] (+ dropped n-term)
    at = Wk.T @ Wq                        # A^T = Wk^T Wq, [q', q]
    w2tilde = Wk.T @ bq                   # [512]
    beta2 = float(bq @ bk)

    a_h = np.ascontiguousarray(at.reshape(4, P, DD)).astype(BF)    # [4,128,512]
    wuv_h = np.zeros((4, P, 33), BF)
    wuv_h[:, :, 0] = wtilde.reshape(4, P).astype(BF)
    wuv_h[:, :, 32] = w2tilde.reshape(4, P).astype(BF)
    betas_h = np.zeros((33, 2), np.float32)
    betas_h[0, 0] = beta
    betas_h[32, 1] = beta2 * SCALE

    in_maps = []
    for b in range(NB):
        # [4, 128, 2048]: chunk-major so each q-chunk is one contiguous DMA
        rt_h = np.ascontiguousarray(R[b].T.reshape(4, P, NN)).astype(BF)
        in_maps.append(
            {
                "rt": rt_h,
                "amat": a_h,
                "wuv": wuv_h,
                "betas": betas_h,
            }
        )

    nc = _get_nc()
    res = run_bass_kernel_spmd(nc, in_maps, core_ids=list(range(N_CORES)))
    outs = np.stack([res.results[b]["out"] for b in range(NB)])   # [8,2,2048]
    return (outs[:, 0] / outs[:, 1] + np.float32(const)).astype(np.float32)



# revision 2
# speedup vs baseline: 1.3640x; 1.3640x over previous
"""CAAN kernel for Trainium2, 8-core data-parallel (one batch row per core).

Math: the reference is
    Q = R Wq^T + bq ; K = R Wk^T + bk ; V = R Wv^T + bv
    E = exp(Q K^T / sqrt(512)) ; saat = E / rowsum(E)
    winner = (saat V) W1^T W2^T + (W2 b1 + b2)

Two algebraic collapses make most of the network disappear:

1. The W1/W2 head is linear, so with c = W1^T W2[0]:
       winner[n] = (sum_m E[n,m] u[m]) / (sum_m E[n,m]) + const,
   u = V c = R (Wv^T c) + bv.c — a per-asset scalar. The V projection and
   attention*V matmul vanish.

2. gamma = Q K^T = R A R^T + (R Wq^T bk)[n] + (R Wk^T bq)[m] + bq.bk with
   A = Wq^T Wk. The per-n term scales E rows uniformly and cancels in the
   s/rowsum ratio, so it is dropped. The per-m term v[m] rides the exp
   activation's per-partition bias slot. The Q and K projections collapse
   into a single projection B = A^T-pack @ R^T.

fp8 version: R, A and the u/v projection weights are quantized to fp8-e4m3
on the host (A and wuv pre-scaled by 64 to clear the e4m3 subnormal range;
the 64 is divided back out in the activation scale factors). All large
matmuls run in DoubleRow perf mode (two 128-row q-chunks contracted per
matmul), which doubles PE throughput. rel-err vs the f32 reference is
~4e-3 (fp8 quantization noise averages out in the softmax sums).

Per-core device work (batch row b):
  uv:     8 DoubleRow matmuls -> u row (psum row 0) and v row (psum row 32),
          written via ACT with bias/scale to undo the x64 and fold 1/sqrt(d).
          u/v rows bounce through DRAM to become [128,16] columns.
  proj:   B[q,m] = sum_q' A[q,q'] R[m,q'] as 32 DoubleRow matmuls into
          [128,1024] 2-bank psum tiles, cast to fp8 bt by DVE.
  gamma:  per 128-row m-chunk: 8 DoubleRow matmuls -> two [128,1024] psum
          tiles; exp(scale*psum + v[m]) -> ET bf16 in two 2-bank ACT ops
          (amortizes ACT per-op overhead; ACT is the near-critical engine).
  srs:    s[n] / rowsum[n] partials via [u|1] weights, col-tiled 4-wide:
          4 m-chunks accumulate concurrently in col-groups j=0..3 at psum
          partitions 32j/32j+1. Rounds trail gamma by one group of 4 chunks
          so the PE never waits on exp.
  out:    8 partial rows DMA'd out [8, 2048] f32; host sums the 4 partials
          for s and rowsum and does winner = s/rowsum + const.
"""

import math

import ml_dtypes
import numpy as np

import concourse.bass as bass
import concourse.mybir as mybir
import concourse.tile as tile
from concourse.bass_utils import run_bass_kernel_spmd
from concourse.vector_clock import ScopedClock


N_CORES = 8
NB, NN, DD = 8, 2048, 512  # batch, assets, feature dim
P = 128
NQ = DD // P   # q chunks (contraction)
NM = NN // P   # m chunks (key/asset rows)
S = 512        # matmul moving free dim / PSUM bank width
NS = NN // S   # n slices of 512
WUVC = 48      # wuv padded col count (16-aligned for DoubleRow weight step)
BF16 = mybir.dt.bfloat16
F32 = mybir.dt.float32
F8 = mybir.dt.float8e4
SCALE = 1.0 / math.sqrt(float(DD))
ASCALE = 64.0  # fp8 pre-scale on A / wuv (keeps entries out of e4m3 subnormals)
BF = ml_dtypes.bfloat16
F8NP = ml_dtypes.float8_e4m3
DR = mybir.MatmulPerfMode.DoubleRow


class _TileContext(tile.TileContext):
    """Workaround for walrus rejecting >1 sem wait on the kernel-tail Drain
    ("Too many sync wait commands"): put each final wait on its own SP NoOp
    ahead of an unwaited Drain."""

    def _drain_and_barrier(self, tick_clock, wait_clock):
        nc = self.nc
        probe = nc.sync.nop(nofuse=True)
        wait_clock.add_sem_waits(
            probe.ins, ScopedClock({None: tick_clock.global_clock})
        )
        si = probe.ins.sync_info
        waits = list(si.on_wait) if si is not None else []
        if si is not None:
            si.on_wait = []
        # spread the final waits round-robin over all engines so they
        # resolve in parallel; the barrier then guarantees every wait has
        # been observed before the SP drain runs.
        engines = [nc.sync, nc.vector, nc.scalar, nc.tensor, nc.gpsimd]
        for i, w in enumerate(waits):
            n = engines[i % len(engines)].nop(nofuse=True)
            n.ins.sync_info = mybir.SyncInfo(on_wait=[w], on_update=[])
        nc.all_engine_barrier()
        nc.sync.drain()
        assert self.sems is not None
        popped = nc._tile_sem_poison_stack.pop()
        assert popped is self._sem_poison
        # clear_and_free_semaphores would range-clear every ALLOCATED sem id
        # (~200+), which walrus lowers to one op per id (~7us of tail).
        # Only ids that appear in the final instruction stream can be
        # non-zero, so hardware-clear just those; do the allocator
        # bookkeeping for the full set.
        allocated = list(self.sems.allocated().values())
        sem_nums = [
            s.num if hasattr(s, "num") else int(s) for s in allocated
        ]
        used = set()
        for fn in nc.m.functions:
            for blk in fn.blocks:
                for inst in blk.instructions:
                    si = inst.sync_info
                    if si is not None:
                        for w in si.on_wait:
                            used.add(w.id)
                        for u in si.on_update:
                            used.add(u.id)
        hw_nums = sorted(n for n in sem_nums if n in used)
        for sem_range in bass.compact_to_ranges(hw_nums):
            nc.gpsimd.dma_reset(sem_range)
            nc.gpsimd.sem_clear(sem_range)
        nc._state.prepend_free_semaphores(sem_nums)
        for poison_set in nc._tile_sem_poison_stack:
            poison_set.update(sem_nums)
        # the trailing all_engine_barrier is skipped: nothing after the
        # clear touches semaphores, and the runtime serializes executions


def _split_multi_waits(nc, maxw=1):
    """This walrus build rejects instructions carrying more than one sync
    wait ("Too many sync wait commands"). Move excess waits onto same-engine
    NoOps inserted just before the instruction: sem-ge waits are monotonic
    within the kernel, so waiting for them earlier on the same engine is
    equivalent. sem-eq waits stay on the original instruction."""
    for fn in nc.m.functions:
        for blk in fn.blocks:
            insts = blk.instructions
            if not any(
                i.sync_info is not None and len(i.sync_info.on_wait) > maxw
                for i in insts
            ):
                continue
            out = []
            for inst in insts:
                si = inst.sync_info
                if si is not None and len(si.on_wait) > maxw:
                    keep = [w for w in si.on_wait if "eq" in w.wait_mode]
                    movable = [w for w in si.on_wait if "eq" not in w.wait_mode]
                    while len(keep) < maxw and movable:
                        keep.append(movable.pop(0))
                    assert len(keep) <= maxw, (
                        f"{inst.name}: {len(keep)} non-splittable waits"
                    )
                    for w in movable:
                        nop = mybir.InstNoOp(
                            name=nc.get_next_instruction_name(), ins=[], outs=[]
                        )
                        nop.engine = inst.engine
                        nop.sync_info = mybir.SyncInfo(on_wait=[w], on_update=[])
                        out.append(nop)
                    si.on_wait = keep
                out.append(inst)
            blk.instructions = out


def _build():
    nc = bass.Bass("TRN2", target_bir_lowering=False, debug=False)

    rt = nc.dram_tensor("rt", (P, NQ, NN), F8, kind="ExternalInput")
    amat = nc.dram_tensor("amat", (P, NQ, DD), F8, kind="ExternalInput")
    wuv = nc.dram_tensor("wuv", (P, NQ, WUVC), F8, kind="ExternalInput")
    betas = nc.dram_tensor("betas", (33, 2), F32, kind="ExternalInput")
    out = nc.dram_tensor("out", (8, NN), F32, kind="ExternalOutput")

    Ident = mybir.ActivationFunctionType.Identity
    Exp = mybir.ActivationFunctionType.Exp

    with _TileContext(nc) as tc:
        with (
            tc.tile_pool(name="const", bufs=1) as cpool,
            tc.tile_pool(name="big", bufs=1) as big,
            tc.tile_pool(name="et", bufs=8) as et_pool,
            tc.tile_pool(name="dscratch", bufs=1, space="DRAM") as dpool,
        ):
            rt_sb = cpool.tile([P, NQ, NN], F8, name="rt")
            a_sb = cpool.tile([P, NQ, DD], F8, name="a")
            wuv_sb = cpool.tile([P, NQ, WUVC], F8, name="wuv")
            betas_sb = cpool.tile([33, 2], F32, name="betas")

            # DMA order matches consumption order: wuv + the first rt
            # pair-halves feed the uv matmuls, a feeds proj wave 0.
            nc.scalar.dma_start(wuv_sb[:], wuv.ap())
            nc.sync.dma_start(
                rt_sb[:, 0:2, 0 : NN // 2], rt.ap()[:, 0:2, 0 : NN // 2]
            )
            nc.gpsimd.dma_start(
                rt_sb[:, 2:4, 0 : NN // 2], rt.ap()[:, 2:4, 0 : NN // 2]
            )
            nc.scalar.dma_start(betas_sb[:], betas.ap())
            nc.sync.dma_start(
                rt_sb[:, 0:2, NN // 2 :], rt.ap()[:, 0:2, NN // 2 :]
            )
            nc.gpsimd.dma_start(a_sb[:], amat.ap())
            nc.sync.dma_start(
                rt_sb[:, 2:4, NN // 2 :], rt.ap()[:, 2:4, NN // 2 :]
            )

            bt_sb = big.tile([P, NQ, NN], F8, name="bt")
            uvrow_sb = big.tile([33, NN], BF16, name="uvrow")
            vcol_sb = big.tile([P, NM], BF16, name="vcol")
            v_sb = big.tile([P, NM], F32, name="v")
            # su columns per m-chunk: 0 = u, 1 = ones. s lands on psum
            # partition 32j, rowsum on 32j+1 for col-group j.
            su_sb = big.tile([P, NM, 2], BF16, name="su")
            nc.vector.memset(su_sb[:, :, 1:2], 1.0)

            # psR: 4 banks for the s/rowsum accumulators (also reused as
            # scratch for the uv projections before srs starts).
            # psMain: 4 banks as two rotating [128,1024] 2-bank tiles so the
            # exp ACTs can cover 1024 columns per instruction.
            psR = tc.alloc_tile_pool(name="psR", bufs=1, space="PSUM")
            psMain = tc.alloc_tile_pool(name="psMain", bufs=2, space="PSUM")
            srs = [
                psR.tile([P, S], F32, tag=f"srs{ns}", name=f"srs{ns}")
                for ns in range(NS)
            ]

            # ---- uv: u (row 0) and v (row 32) from the [wtl|w2tl] lhsT ----
            for ns in range(NS):
                pur = srs[ns][0:WUVC, :]
                for pr in range(2):
                    nc.tensor.matmul(
                        pur[:],
                        wuv_sb[:, 2 * pr : 2 * pr + 2, :],
                        rt_sb[:, 2 * pr : 2 * pr + 2, ns * S : (ns + 1) * S],
                        start=(pr == 0),
                        stop=(pr == 1),
                        perf_mode=DR,
                        skip_group_check=True,
                    )
                nc.scalar.activation(
                    uvrow_sb[0:1, ns * S : (ns + 1) * S],
                    pur[0:1, :],
                    Ident,
                    bias=betas_sb[0:1, 0:1],
                    scale=1.0 / ASCALE,
                )
                nc.scalar.activation(
                    uvrow_sb[32:33, ns * S : (ns + 1) * S],
                    pur[32:33, :],
                    Ident,
                    bias=betas_sb[32:33, 1:2],
                    scale=SCALE / ASCALE,
                )

            # scatter rows [1, 2048] -> columns [128, 16] off the PE:
            # bounce through flat DRAM, where the partition-scatter read
            # pattern is expressible.
            uv_dram = dpool.tile([2, NN], BF16)
            nc.sync.dma_start(uv_dram[0:1, :], uvrow_sb[0:1, :])
            nc.sync.dma_start(uv_dram[1:2, :], uvrow_sb[32:33, :])
            with nc.allow_non_contiguous_dma(
                reason="2048-elem partition scatter, one-off"
            ):
                nc.sync.dma_start(
                    su_sb[:, :, 0],
                    uv_dram[0, :].rearrange("(m p) -> p m", p=P),
                )
                nc.sync.dma_start(
                    vcol_sb[:],
                    uv_dram[1, :].rearrange("(m p) -> p m", p=P),
                )
            nc.vector.tensor_copy(v_sb[:], vcol_sb[:])

            # ---- proj: B = A R^T, DoubleRow, cast to fp8 by DVE ----
            for qo in range(NQ):
                for half in range(2):
                    t = psMain.tile([P, 2 * S], F32, tag="mm", name="mm")
                    for sub in range(2):
                        ns = 2 * half + sub
                        po = t[:, sub * S : (sub + 1) * S]
                        for pr in range(2):
                            nc.tensor.matmul(
                                po[:],
                                a_sb[:, 2 * pr : 2 * pr + 2, qo * P : (qo + 1) * P],
                                rt_sb[:, 2 * pr : 2 * pr + 2, ns * S : (ns + 1) * S],
                                start=(pr == 0),
                                stop=(pr == 1),
                                perf_mode=DR,
                            )
                    nc.vector.tensor_copy(
                        bt_sb[:, qo, half * 2 * S : (half + 1) * 2 * S], t[:]
                    )

            # ---- gamma + exp + s/rowsum ----
            ets = {}

            def gamma(mc):
                et = et_pool.tile([P, NN], BF16, tag="et", name="et")
                ets[mc] = et
                for half in range(2):
                    g = psMain.tile([P, 2 * S], F32, tag="mm", name="mm")
                    for sub in range(2):
                        ns = 2 * half + sub
                        go = g[:, sub * S : (sub + 1) * S]
                        for pr in range(2):
                            nc.tensor.matmul(
                                go[:],
                                bt_sb[:, 2 * pr : 2 * pr + 2, mc * P : (mc + 1) * P],
                                rt_sb[:, 2 * pr : 2 * pr + 2, ns * S : (ns + 1) * S],
                                start=(pr == 0),
                                stop=(pr == 1),
                                perf_mode=DR,
                            )
                    nc.scalar.activation(
                        et[:, half * 2 * S : (half + 1) * 2 * S],
                        g[:],
                        Exp,
                        bias=v_sb[:, mc : mc + 1],
                        scale=SCALE / ASCALE,
                    )

            def srs_round(grp):
                # 4 m-chunks accumulate concurrently in col-groups j=0..3
                for ns in range(NS):
                    for j in range(4):
                        mc = 4 * grp + j
                        nc.tensor.matmul(
                            srs[ns][32 * j : 32 * j + 2, :],
                            su_sb[:, mc, :],
                            ets[mc][:, ns * S : (ns + 1) * S],
                            start=(grp == 0),
                            stop=(grp == 3),
                            skip_group_check=True,
                            tile_position=(0, 32 * j),
                        )

            # srs rounds trail gamma by one group of 4 chunks so the PE
            # never stalls on the exp activations.
            for mc in range(8):
                gamma(mc)
            srs_round(0)
            for mc in range(8, 12):
                gamma(mc)
            srs_round(1)
            for mc in range(12, 16):
                gamma(mc)
            srs_round(2)
            srs_round(3)

            # copy PSUM -> SBUF (partitions 0..97 cover rows 32j/32j+1),
            # then DMA the 8 partial rows out.
            out_sb = big.tile([98, NN], F32, name="out_sb")
            for ns in range(NS):
                sl = slice(ns * S, (ns + 1) * S)
                # alternate DVE/ACT so the four drain copies run on two
                # engines in parallel
                if ns % 2 == 0:
                    nc.vector.tensor_copy(out_sb[:, sl], srs[ns][0:98, :])
                else:
                    nc.scalar.copy(out_sb[:, sl], srs[ns][0:98, :])
            for j in range(4):
                eng = [nc.sync, nc.gpsimd, nc.sync, nc.gpsimd][j]
                eng.dma_start(
                    out.ap()[2 * j : 2 * j + 2, :],
                    out_sb[32 * j : 32 * j + 2, :],
                )
            psMain.release()
            psR.release()

    _split_multi_waits(nc)
    return nc


_NC = None


def _get_nc():
    global _NC
    if _NC is None:
        _NC = _build()
    return _NC


def kernel(R, Wq, bq, Wk, bk, Wv, bv, W1, b1, W2, b2):
    R = np.asarray(R, np.float32)
    Wq = np.asarray(Wq, np.float64)
    bq = np.asarray(bq, np.float64)
    Wk = np.asarray(Wk, np.float64)
    bk = np.asarray(bk, np.float64)
    Wv = np.asarray(Wv, np.float64)
    bv = np.asarray(bv, np.float64)
    W1 = np.asarray(W1, np.float64)
    b1 = np.asarray(b1, np.float64)
    W2 = np.asarray(W2, np.float64)
    b2 = np.asarray(b2, np.float64)

    # Collapse the linear head: winner = c.a + const, u = V c.
    c = W1.T @ W2[0]                      # [512]
    wtilde = Wv.T @ c                     # [512]
    beta = float(bv @ c)
    const = float(W2[0] @ b1 + b2[0])
    # Collapse the Q/K projections: gamma = R A R^T + v[m] (+ dropped n-term)
    at = Wk.T @ Wq                        # A^T = Wk^T Wq, [q', q]
    w2tilde = Wk.T @ bq                   # [512]
    beta2 = float(bq @ bk)

    # [p, chunk, d] with q = chunk*128 + p; x64 pre-scale for fp8 range
    a_h = np.ascontiguousarray(
        (at * ASCALE).reshape(NQ, P, DD).transpose(1, 0, 2)
    ).astype(F8NP)
    wuv_h = np.zeros((P, NQ, WUVC), F8NP)
    wuv_h[:, :, 0] = (wtilde * ASCALE).reshape(NQ, P).T.astype(F8NP)
    wuv_h[:, :, 32] = (w2tilde * ASCALE).reshape(NQ, P).T.astype(F8NP)
    betas_h = np.zeros((33, 2), np.float32)
    betas_h[0, 0] = beta
    betas_h[32, 1] = beta2 * SCALE

    in_maps = []
    for b in range(NB):
        # [p, chunk, n]: R[b].T chunked over q so each q-chunk-pair slice
        # is a regular strided DMA
        rt_h = np.ascontiguousarray(
            R[b].T.reshape(NQ, P, NN).transpose(1, 0, 2)
        ).astype(F8NP)
        in_maps.append(
            {
                "rt": rt_h,
                "amat": a_h,
                "wuv": wuv_h,
                "betas": betas_h,
            }
        )

    nc = _get_nc()
    res = run_bass_kernel_spmd(nc, in_maps, core_ids=list(range(N_CORES)))
    outs = np.stack([res.results[b]["out"] for b in range(NB)])   # [8,8,2048]
    s = outs[:, 0::2, :].sum(axis=1)
    rs = outs[:, 1::2, :].sum(axis=1)
    return (s / rs + np.float32(const)).astype(np.float32)


# revision 6
# speedup vs baseline: 1.4708x; 1.0783x over previous
"""CAAN kernel for Trainium2, 8-core data-parallel (one batch row per core).

Math: the reference is
    Q = R Wq^T + bq ; K = R Wk^T + bk ; V = R Wv^T + bv
    E = exp(Q K^T / sqrt(512)) ; saat = E / rowsum(E)
    winner = (saat V) W1^T W2^T + (W2 b1 + b2)

Two algebraic collapses make most of the network disappear:

1. The W1/W2 head is linear, so with c = W1^T W2[0]:
       winner[n] = (sum_m E[n,m] u[m]) / (sum_m E[n,m]) + const,
   u = V c = R (Wv^T c) + bv.c — a per-asset scalar. The V projection and
   attention*V matmul vanish.

2. gamma = Q K^T = R A R^T + (R Wq^T bk)[n] + (R Wk^T bq)[m] + bq.bk with
   A = Wq^T Wk. The per-n term scales E rows uniformly and cancels in the
   s/rowsum ratio, so it is dropped. The per-m term v[m] rides the exp
   activation's per-partition bias slot. The Q and K projections collapse
   into a single projection B = A^T-pack @ R^T.

fp8 version: everything the PE touches is fp8-e4m3 (A and the u/v weights
pre-scaled by 64 to clear the e4m3 subnormal range; the 64 is divided back
out in the activation scale factors). All big matmuls run in DoubleRow
perf mode (256 contraction rows per matmul) for 2x PE throughput; the
s/rowsum reduction is DoubleRow too (exp output ET is fp8, two m-chunks
per matmul). rel-err vs the f32 reference ~5e-3: fp8 quantization noise
averages out in the softmax sums.

Pipeline (per core, batch row b), ordered to keep both PE and ACT >90%
busy (ACT exp at ~1.1us per [128,1024] 2-bank tile is the near-critical
engine):
  warmup: 5 dummy matmuls un-throttle the PE HAM clock gate during the
          input-DMA ramp.
  uv:     8 DoubleRow matmuls; ONE fused ACT per 512-slice writes u (row 0,
          scale 1/64 + beta) and v (row 32, scale SCALE/64 + beta2*SCALE)
          via per-partition scale/bias APs. Rows bounce through DRAM to
          become [128, pair, sub] fp8 columns.
  proj:   B = A R^T, DoubleRow, qo-waves split in two n-halves; PSUM->fp8
          bt casts alternate DVE/GpSimd so the cast chain never gates the
          PE. gamma chunks 0-7 run right after the first n-half.
  gamma:  per m-chunk: 8 DoubleRow matmuls into two [128,1024] 2-bank psum
          tiles; exp(scale*psum + v[m]) -> fp8 ET in two 2-bank ACT ops.
  srs:    s (row 0) and rowsum (row 1) accumulate over 8 chunk-pairs of
          DoubleRow matmuls with [u|1|0...] weights, trailing gamma so the
          PE never waits on exp.
  out:    [2, 2048] f32 DMA'd out; host does winner = s/rowsum + const.
"""

import math

import ml_dtypes
import numpy as np

import concourse.bass as bass
import concourse.mybir as mybir
import concourse.tile as tile
from concourse.bass_utils import run_bass_kernel_spmd
from concourse.vector_clock import ScopedClock


N_CORES = 8
NB, NN, DD = 8, 2048, 512  # batch, assets, feature dim
P = 128
NQ = DD // P   # q chunks (contraction)
NM = NN // P   # m chunks (key/asset rows)
NPR = NM // 2  # m chunk-pairs for the DoubleRow s/rowsum reduction
S = 512        # PSUM bank width in f32
NS = NN // S   # n slices of 512
WUVC = 48      # wuv padded col count (16-aligned for DoubleRow weight step)
SUC = 16       # su padded col count (16-aligned DoubleRow weight step)
BF16 = mybir.dt.bfloat16
F32 = mybir.dt.float32
F8 = mybir.dt.float8e4
SCALE = 1.0 / math.sqrt(float(DD))
ASCALE = 64.0  # fp8 pre-scale on A / wuv (keeps entries out of e4m3 subnormals)
BF = ml_dtypes.bfloat16
F8NP = ml_dtypes.float8_e4m3
DR = mybir.MatmulPerfMode.DoubleRow


class _TileContext(tile.TileContext):
    """Workaround for walrus rejecting >1 sem wait on the kernel-tail Drain
    ("Too many sync wait commands"): put each final wait on its own SP NoOp
    ahead of an unwaited Drain."""

    def _drain_and_barrier(self, tick_clock, wait_clock):
        nc = self.nc
        probe = nc.sync.nop(nofuse=True)
        wait_clock.add_sem_waits(
            probe.ins, ScopedClock({None: tick_clock.global_clock})
        )
        si = probe.ins.sync_info
        waits = list(si.on_wait) if si is not None else []
        if si is not None:
            si.on_wait = []
        # spread the final waits round-robin over all engines so they
        # resolve in parallel; the barrier then guarantees every wait has
        # been observed before the SP drain runs.
        engines = [nc.sync, nc.vector, nc.scalar, nc.tensor, nc.gpsimd]
        for i, w in enumerate(waits):
            n = engines[i % len(engines)].nop(nofuse=True)
            n.ins.sync_info = mybir.SyncInfo(on_wait=[w], on_update=[])
        nc.all_engine_barrier()
        nc.sync.drain()
        assert self.sems is not None
        popped = nc._tile_sem_poison_stack.pop()
        assert popped is self._sem_poison
        # clear_and_free_semaphores would range-clear every ALLOCATED sem id
        # (~200+), which walrus lowers to one op per id (~7us of tail).
        # Only ids that appear in the final instruction stream can be
        # non-zero, so hardware-clear just those; do the allocator
        # bookkeeping for the full set.
        allocated = list(self.sems.allocated().values())
        sem_nums = [
            s.num if hasattr(s, "num") else int(s) for s in allocated
        ]
        used = set()
        for fn in nc.m.functions:
            for blk in fn.blocks:
                for inst in blk.instructions:
                    si = inst.sync_info
                    if si is not None:
                        for w in si.on_wait:
                            used.add(w.id)
                        for u in si.on_update:
                            used.add(u.id)
        hw_nums = sorted(n for n in sem_nums if n in used)
        for sem_range in bass.compact_to_ranges(hw_nums):
            nc.gpsimd.dma_reset(sem_range)
            nc.gpsimd.sem_clear(sem_range)
        nc._state.prepend_free_semaphores(sem_nums)
        for poison_set in nc._tile_sem_poison_stack:
            poison_set.update(sem_nums)
        # the trailing all_engine_barrier is skipped: nothing after the
        # clear touches semaphores, and the runtime serializes executions


def _split_multi_waits(nc, maxw=1):
    """This walrus build rejects instructions carrying more than one sync
    wait ("Too many sync wait commands"). Move excess waits onto same-engine
    NoOps inserted just before the instruction: sem-ge waits are monotonic
    within the kernel, so waiting for them earlier on the same engine is
    equivalent. sem-eq waits stay on the original instruction."""
    for fn in nc.m.functions:
        for blk in fn.blocks:
            insts = blk.instructions
            if not any(
                i.sync_info is not None and len(i.sync_info.on_wait) > maxw
                for i in insts
            ):
                continue
            out = []
            for inst in insts:
                si = inst.sync_info
                if si is not None and len(si.on_wait) > maxw:
                    keep = [w for w in si.on_wait if "eq" in w.wait_mode]
                    movable = [w for w in si.on_wait if "eq" not in w.wait_mode]
                    while len(keep) < maxw and movable:
                        keep.append(movable.pop(0))
                    assert len(keep) <= maxw, (
                        f"{inst.name}: {len(keep)} non-splittable waits"
                    )
                    for w in movable:
                        nop = mybir.InstNoOp(
                            name=nc.get_next_instruction_name(), ins=[], outs=[]
                        )
                        nop.engine = inst.engine
                        nop.sync_info = mybir.SyncInfo(on_wait=[w], on_update=[])
                        out.append(nop)
                    si.on_wait = keep
                out.append(inst)
            blk.instructions = out


def _build():
    nc = bass.Bass("TRN2", target_bir_lowering=False, debug=False)

    rt = nc.dram_tensor("rt", (P, NQ, NN), F8, kind="ExternalInput")
    amat = nc.dram_tensor("amat", (P, NQ, DD), F8, kind="ExternalInput")
    wuv = nc.dram_tensor("wuv", (P, NQ, WUVC), F8, kind="ExternalInput")
    betas = nc.dram_tensor("betas", (33, 2), F32, kind="ExternalInput")
    out = nc.dram_tensor("out", (2, NN), F32, kind="ExternalOutput")

    Ident = mybir.ActivationFunctionType.Identity
    Exp = mybir.ActivationFunctionType.Exp

    with _TileContext(nc) as tc:
        with (
            tc.tile_pool(name="const", bufs=1) as cpool,
            tc.tile_pool(name="big", bufs=1) as big,
            tc.tile_pool(name="et", bufs=6) as et_pool,
            tc.tile_pool(name="dscratch", bufs=1, space="DRAM") as dpool,
        ):
            rt_sb = cpool.tile([P, NQ, NN], F8, name="rt")
            a_sb = cpool.tile([P, NQ, DD], F8, name="a")
            wuv_sb = cpool.tile([P, NQ, WUVC], F8, name="wuv")
            betas_sb = cpool.tile([33, 2], F32, name="betas")
            dummy_sb = cpool.tile([P, S], BF16, name="dummy")

            # DMA order matches consumption order: wuv + the first rt
            # pair-slices feed the uv matmuls, a feeds proj wave 0.
            nc.scalar.dma_start(wuv_sb[:], wuv.ap())
            nc.scalar.dma_start(betas_sb[:], betas.ap())
            nc.scalar.dma_start(a_sb[:], amat.ap())
            for q in range(NS):
                sl = slice(q * S, (q + 1) * S)
                nc.sync.dma_start(rt_sb[:, 0:2, sl], rt.ap()[:, 0:2, sl])
                nc.gpsimd.dma_start(rt_sb[:, 2:4, sl], rt.ap()[:, 2:4, sl])

            bt_sb = big.tile([P, NQ, NN], F8, name="bt")
            uvrow_sb = big.tile([33, NN], F8, name="uvrow")
            vcol_sb = big.tile([P, NM], F8, name="vcol")
            v_sb = big.tile([P, NM], F32, name="v")
            # su weights per m-chunk-pair: [pair, sub, col] with col 0 = u,
            # col 1 = ones, cols 2..15 zero padding (16B DoubleRow step).
            su_sb = big.tile([P, NPR, 2, SUC], F8, name="su")
            nc.vector.memset(su_sb[:], 0.0)
            nc.vector.memset(su_sb[:, :, :, 1:2], 1.0)

            # psR: 4 banks for the s/rowsum accumulators (also reused as
            # scratch for the uv projections before srs starts).
            # psMain: 4 banks as two rotating [128,1024] 2-bank tiles so the
            # exp ACTs can cover 1024 columns per instruction.
            psR = tc.alloc_tile_pool(name="psR", bufs=1, space="PSUM")
            psMain = tc.alloc_tile_pool(name="psMain", bufs=2, space="PSUM")
            srs = [
                psR.tile([P, S], F32, tag=f"srs{ns}", name=f"srs{ns}")
                for ns in range(NS)
            ]

            # ---- PE warmup: dummy matmuls (no input deps) un-throttle the
            # HAM clock gate while the input DMAs stream in.
            nc.vector.memset(dummy_sb[:], 0.0)
            for _ in range(5):
                t = psMain.tile([P, 2 * S], F32, tag="mm", name="mm")
                nc.tensor.matmul(
                    t[:, 0:S], dummy_sb[:, 0:P], dummy_sb[:], start=True, stop=True
                )

            # ---- uv: u (row 0) and v (row 32) from the [wtl|w2tl] lhsT;
            # one fused ACT per slice via per-partition scale/bias columns.
            for ns in range(NS):
                pur = srs[ns][0:WUVC, :]
                for pr in range(2):
                    nc.tensor.matmul(
                        pur[:],
                        wuv_sb[:, 2 * pr : 2 * pr + 2, :],
                        rt_sb[:, 2 * pr : 2 * pr + 2, ns * S : (ns + 1) * S],
                        start=(pr == 0),
                        stop=(pr == 1),
                        perf_mode=DR,
                        skip_group_check=True,
                    )
                nc.scalar.activation(
                    uvrow_sb[0:33, ns * S : (ns + 1) * S],
                    pur[0:33, :],
                    Ident,
                    bias=betas_sb[0:33, 1:2],
                    scale=betas_sb[0:33, 0:1],
                )

            # scatter rows [1, 2048] -> [128, pair, sub] columns off the PE:
            # bounce through flat DRAM, where the partition-scatter read
            # pattern is expressible.
            uv_dram = dpool.tile([2, NN], F8)
            nc.sync.dma_start(uv_dram[0:1, :], uvrow_sb[0:1, :])
            nc.sync.dma_start(uv_dram[1:2, :], uvrow_sb[32:33, :])
            with nc.allow_non_contiguous_dma(
                reason="2048-elem partition scatter, one-off"
            ):
                nc.sync.dma_start(
                    su_sb[:, :, :, 0],
                    uv_dram[0, :].rearrange("(pr i p) -> p pr i", p=P, i=2),
                )
                nc.sync.dma_start(
                    vcol_sb[:],
                    uv_dram[1, :].rearrange("(m p) -> p m", p=P),
                )
            # gpsimd, not DVE: this copy waits on the scatter DMA and must
            # not block the DVE bt-cast chain behind it
            nc.gpsimd.tensor_copy(v_sb[:], vcol_sb[:])

            # ---- proj: B = A R^T, DoubleRow, cast to fp8 bt ----
            def proj(qo, half):
                t = psMain.tile([P, 2 * S], F32, tag="mm", name="mm")
                for sub in range(2):
                    ns = 2 * half + sub
                    po = t[:, sub * S : (sub + 1) * S]
                    for pr in range(2):
                        nc.tensor.matmul(
                            po[:],
                            a_sb[:, 2 * pr : 2 * pr + 2, qo * P : (qo + 1) * P],
                            rt_sb[:, 2 * pr : 2 * pr + 2, ns * S : (ns + 1) * S],
                            start=(pr == 0),
                            stop=(pr == 1),
                            perf_mode=DR,
                        )
                nc.vector.tensor_copy(
                    bt_sb[:, qo, half * 2 * S : (half + 1) * 2 * S], t[:]
                )

            # ---- gamma + exp ----
            ets = {}

            def gamma(mc):
                if mc % 2 == 0:
                    ets[mc // 2] = et_pool.tile(
                        [P, 2, NN], F8, tag="et", name="et"
                    )
                et = ets[mc // 2]
                for half in range(2):
                    g = psMain.tile([P, 2 * S], F32, tag="mm", name="mm")
                    for sub in range(2):
                        ns = 2 * half + sub
                        go = g[:, sub * S : (sub + 1) * S]
                        for pr in range(2):
                            nc.tensor.matmul(
                                go[:],
                                bt_sb[:, 2 * pr : 2 * pr + 2, mc * P : (mc + 1) * P],
                                rt_sb[:, 2 * pr : 2 * pr + 2, ns * S : (ns + 1) * S],
                                start=(pr == 0),
                                stop=(pr == 1),
                                perf_mode=DR,
                            )
                    nc.scalar.activation(
                        et[:, mc % 2, half * 2 * S : (half + 1) * 2 * S],
                        g[:],
                        Exp,
                        bias=v_sb[:, mc : mc + 1],
                        scale=SCALE / ASCALE,
                    )

            # ---- srs: s/rowsum accumulation over m chunk-pairs ----
            def srs_pair(pp):
                et = ets[pp]
                for ns in range(NS):
                    nc.tensor.matmul(
                        srs[ns][0:SUC, :],
                        su_sb[:, pp, :, :],
                        et[:, :, ns * S : (ns + 1) * S],
                        start=(pp == 0),
                        stop=(pp == NPR - 1),
                        perf_mode=DR,
                        skip_group_check=True,
                    )
                del ets[pp]

            # schedule: gamma chunks 0-7 run right after the first proj
            # n-half so the exp stream (the critical ACT work) starts early;
            # srs pairs trail their gamma chunks by >=1 full chunk.
            for qo in range(NQ):
                proj(qo, 0)
            proj(0, 1)
            for mc in range(0, 8):
                gamma(mc)
            for qo in range(1, NQ):
                proj(qo, 1)
            srs_pair(0)
            srs_pair(1)
            for mc in range(8, 12):
                gamma(mc)
            srs_pair(2)
            srs_pair(3)
            for mc in range(12, 16):
                gamma(mc)
            srs_pair(4)
            srs_pair(5)
            srs_pair(6)
            srs_pair(7)

            # copy the two result rows PSUM -> SBUF, then one DMA out.
            out_sb = big.tile([2, NN], F32, name="out_sb")
            for ns in range(NS):
                sl = slice(ns * S, (ns + 1) * S)
                # alternate DVE/ACT so the four drain copies run on two
                # engines in parallel
                if ns % 2 == 0:
                    nc.vector.tensor_copy(out_sb[:, sl], srs[ns][0:2, :])
                else:
                    nc.scalar.copy(out_sb[:, sl], srs[ns][0:2, :])
            nc.sync.dma_start(out.ap()[:], out_sb[:])
            psMain.release()
            psR.release()

    _split_multi_waits(nc)
    return nc


_NC = None


def _get_nc():
    global _NC
    if _NC is None:
        _NC = _build()
    return _NC


def kernel(R, Wq, bq, Wk, bk, Wv, bv, W1, b1, W2, b2):
    R = np.asarray(R, np.float32)
    Wq = np.asarray(Wq, np.float64)
    bq = np.asarray(bq, np.float64)
    Wk = np.asarray(Wk, np.float64)
    bk = np.asarray(bk, np.float64)
    Wv = np.asarray(Wv, np.float64)
    bv = np.asarray(bv, np.float64)
    W1 = np.asarray(W1, np.float64)
    b1 = np.asarray(b1, np.float64)
    W2 = np.asarray(W2, np.float64)
    b2 = np.asarray(b2, np.float64)

    # Collapse the linear head: winner = c.a + const, u = V c.
    c = W1.T @ W2[0]                      # [512]
    wtilde = Wv.T @ c                     # [512]
    beta = float(bv @ c)
    const = float(W2[0] @ b1 + b2[0])
    # Collapse the Q/K projections: gamma = R A R^T + v[m] (+ dropped n-term)
    at = Wk.T @ Wq                        # A^T = Wk^T Wq, [q', q]
    w2tilde = Wk.T @ bq                   # [512]
    beta2 = float(bq @ bk)

    # [p, chunk, d] with q = chunk*128 + p; x64 pre-scale for fp8 range
    a_h = np.ascontiguousarray(
        (at * ASCALE).reshape(NQ, P, DD).transpose(1, 0, 2)
    ).astype(F8NP)
    wuv_h = np.zeros((P, NQ, WUVC), F8NP)
    wuv_h[:, :, 0] = (wtilde * ASCALE).reshape(NQ, P).T.astype(F8NP)
    wuv_h[:, :, 32] = (w2tilde * ASCALE).reshape(NQ, P).T.astype(F8NP)
    # fused uv activation: per-partition (scale, bias) columns
    betas_h = np.zeros((33, 2), np.float32)
    betas_h[0] = (1.0 / ASCALE, beta)
    betas_h[32] = (SCALE / ASCALE, beta2 * SCALE)

    in_maps = []
    for b in range(NB):
        # [p, chunk, n]: R[b].T chunked over q so each q-chunk-pair slice
        # is a regular strided DMA
        rt_h = np.ascontiguousarray(
            R[b].T.reshape(NQ, P, NN).transpose(1, 0, 2)
        ).astype(F8NP)
        in_maps.append(
            {
                "rt": rt_h,
                "amat": a_h,
                "wuv": wuv_h,
                "betas": betas_h,
            }
        )

    nc = _get_nc()
    res = run_bass_kernel_spmd(nc, in_maps, core_ids=list(range(N_CORES)))
    outs = np.stack([res.results[b]["out"] for b in range(NB)])   # [8,2,2048]
    return (outs[:, 0] / outs[:, 1] + np.float32(const)).astype(np.float32)


# revision 8
# speedup vs baseline: 1.5185x; 1.0324x over previous
"""CAAN kernel for Trainium2, 8-core data-parallel (one batch row per core).

Math: the reference is
    Q = R Wq^T + bq ; K = R Wk^T + bk ; V = R Wv^T + bv
    E = exp(Q K^T / sqrt(512)) ; saat = E / rowsum(E)
    winner = (saat V) W1^T W2^T + (W2 b1 + b2)

Two algebraic collapses make most of the network disappear:

1. The W1/W2 head is linear, so with c = W1^T W2[0]:
       winner[n] = (sum_m E[n,m] u[m]) / (sum_m E[n,m]) + const,
   u = V c = R (Wv^T c) + bv.c — a per-asset scalar. The V projection and
   attention*V matmul vanish.

2. gamma = Q K^T = R A R^T + (R Wq^T bk)[n] + (R Wk^T bq)[m] + bq.bk with
   A = Wq^T Wk. The per-n term scales E rows uniformly and cancels in the
   s/rowsum ratio, so it is dropped. The per-m term v[m] rides the exp
   activation's per-partition bias slot. The Q and K projections collapse
   into a single projection B = A^T-pack @ R^T.

fp8 version: everything the PE touches is fp8-e4m3 (A and the u/v weights
pre-scaled by 64 to clear the e4m3 subnormal range; the 64 is divided back
out in the activation scale factors). All big matmuls run in DoubleRow
perf mode (256 contraction rows per matmul) for 2x PE throughput; the
s/rowsum reduction is DoubleRow too (exp output ET is fp8, two m-chunks
per matmul). rel-err vs the f32 reference ~5e-3: fp8 quantization noise
averages out in the softmax sums.

Pipeline (per core, batch row b), ordered to keep both PE and ACT >90%
busy (ACT exp at ~1.1us per [128,1024] 2-bank tile is the near-critical
engine):
  warmup: 5 dummy matmuls un-throttle the PE HAM clock gate during the
          input-DMA ramp.
  uv:     8 DoubleRow matmuls; ONE fused ACT per 512-slice writes u (row 0,
          scale 1/64 + beta) and v (row 32, scale SCALE/64 + beta2*SCALE)
          via per-partition scale/bias APs. Rows bounce through DRAM to
          become [128, pair, sub] fp8 columns.
  proj:   B = A R^T, DoubleRow, qo-waves split in two n-halves; PSUM->fp8
          bt casts alternate DVE/GpSimd so the cast chain never gates the
          PE. gamma chunks 0-7 run right after the first n-half.
  gamma:  per m-chunk: 8 DoubleRow matmuls into two [128,1024] 2-bank psum
          tiles; exp(scale*psum + v[m]) -> fp8 ET in two 2-bank ACT ops.
  srs:    s (row 0) and rowsum (row 1) accumulate over 8 chunk-pairs of
          DoubleRow matmuls with [u|1|0...] weights, trailing gamma so the
          PE never waits on exp.
  out:    [2, 2048] f32 DMA'd out; host does winner = s/rowsum + const.
"""

import math

import ml_dtypes
import numpy as np

import concourse.bass as bass
import concourse.mybir as mybir
import concourse.tile as tile
from concourse.bass_utils import run_bass_kernel_spmd
from concourse.vector_clock import ScopedClock


N_CORES = 8
NB, NN, DD = 8, 2048, 512  # batch, assets, feature dim
P = 128
NQ = DD // P   # q chunks (contraction)
NM = NN // P   # m chunks (key/asset rows)
NPR = NM // 2  # m chunk-pairs for the DoubleRow s/rowsum reduction
S = 512        # PSUM bank width in f32
NS = NN // S   # n slices of 512
WUVC = 48      # wuv padded col count (16-aligned for DoubleRow weight step)
SUC = 16       # su padded col count (16-aligned DoubleRow weight step)
BF16 = mybir.dt.bfloat16
F32 = mybir.dt.float32
F8 = mybir.dt.float8e4
SCALE = 1.0 / math.sqrt(float(DD))
ASCALE = 64.0  # fp8 pre-scale on A / wuv (keeps entries out of e4m3 subnormals)
BF = ml_dtypes.bfloat16
F8NP = ml_dtypes.float8_e4m3
DR = mybir.MatmulPerfMode.DoubleRow


class _TileContext(tile.TileContext):
    """Workaround for walrus rejecting >1 sem wait on the kernel-tail Drain
    ("Too many sync wait commands"): put each final wait on its own SP NoOp
    ahead of an unwaited Drain."""

    def _drain_and_barrier(self, tick_clock, wait_clock):
        nc = self.nc
        probe = nc.sync.nop(nofuse=True)
        wait_clock.add_sem_waits(
            probe.ins, ScopedClock({None: tick_clock.global_clock})
        )
        si = probe.ins.sync_info
        waits = list(si.on_wait) if si is not None else []
        if si is not None:
            si.on_wait = []
        # spread the final waits round-robin over all engines so they
        # resolve in parallel; the barrier then guarantees every wait has
        # been observed before the SP drain runs.
        engines = [nc.sync, nc.vector, nc.scalar, nc.tensor, nc.gpsimd]
        for i, w in enumerate(waits):
            n = engines[i % len(engines)].nop(nofuse=True)
            n.ins.sync_info = mybir.SyncInfo(on_wait=[w], on_update=[])
        nc.all_engine_barrier()
        nc.sync.drain()
        assert self.sems is not None
        popped = nc._tile_sem_poison_stack.pop()
        assert popped is self._sem_poison
        # clear_and_free_semaphores would range-clear every ALLOCATED sem id
        # (~200+), which walrus lowers to one op per id (~7us of tail).
        # Only ids that appear in the final instruction stream can be
        # non-zero, so hardware-clear just those; do the allocator
        # bookkeeping for the full set.
        allocated = list(self.sems.allocated().values())
        sem_nums = [
            s.num if hasattr(s, "num") else int(s) for s in allocated
        ]
        used = set()
        for fn in nc.m.functions:
            for blk in fn.blocks:
                for inst in blk.instructions:
                    si = inst.sync_info
                    if si is not None:
                        for w in si.on_wait:
                            used.add(w.id)
                        for u in si.on_update:
                            used.add(u.id)
        hw_nums = sorted(n for n in sem_nums if n in used)
        for sem_range in bass.compact_to_ranges(hw_nums):
            nc.gpsimd.dma_reset(sem_range)
            nc.gpsimd.sem_clear(sem_range)
        nc._state.prepend_free_semaphores(sem_nums)
        for poison_set in nc._tile_sem_poison_stack:
            poison_set.update(sem_nums)
        # the trailing all_engine_barrier is skipped: nothing after the
        # clear touches semaphores, and the runtime serializes executions


def _split_multi_waits(nc, maxw=1):
    """This walrus build rejects instructions carrying more than one sync
    wait ("Too many sync wait commands"). Move excess waits onto same-engine
    NoOps inserted just before the instruction: sem-ge waits are monotonic
    within the kernel, so waiting for them earlier on the same engine is
    equivalent. sem-eq waits stay on the original instruction."""
    for fn in nc.m.functions:
        for blk in fn.blocks:
            insts = blk.instructions
            if not any(
                i.sync_info is not None and len(i.sync_info.on_wait) > maxw
                for i in insts
            ):
                continue
            out = []
            for inst in insts:
                si = inst.sync_info
                if si is not None and len(si.on_wait) > maxw:
                    keep = [w for w in si.on_wait if "eq" in w.wait_mode]
                    movable = [w for w in si.on_wait if "eq" not in w.wait_mode]
                    while len(keep) < maxw and movable:
                        keep.append(movable.pop(0))
                    assert len(keep) <= maxw, (
                        f"{inst.name}: {len(keep)} non-splittable waits"
                    )
                    for w in movable:
                        nop = mybir.InstNoOp(
                            name=nc.get_next_instruction_name(), ins=[], outs=[]
                        )
                        nop.engine = inst.engine
                        nop.sync_info = mybir.SyncInfo(on_wait=[w], on_update=[])
                        out.append(nop)
                    si.on_wait = keep
                out.append(inst)
            blk.instructions = out


def _build():
    nc = bass.Bass("TRN2", target_bir_lowering=False, debug=False)

    rt = nc.dram_tensor("rt", (P, NQ, NN), F8, kind="ExternalInput")
    amat = nc.dram_tensor("amat", (P, NQ, DD), F8, kind="ExternalInput")
    wuv = nc.dram_tensor("wuv", (P, NQ, WUVC), F8, kind="ExternalInput")
    betas = nc.dram_tensor("betas", (33, 2), F32, kind="ExternalInput")
    out = nc.dram_tensor("out", (2, NN), F32, kind="ExternalOutput")

    Ident = mybir.ActivationFunctionType.Identity
    Exp = mybir.ActivationFunctionType.Exp

    with _TileContext(nc) as tc:
        with (
            tc.tile_pool(name="const", bufs=1) as cpool,
            tc.tile_pool(name="big", bufs=1) as big,
            tc.tile_pool(name="et", bufs=6) as et_pool,
            tc.tile_pool(name="dscratch", bufs=1, space="DRAM") as dpool,
        ):
            rt_sb = cpool.tile([P, NQ, NN], F8, name="rt")
            a_sb = cpool.tile([P, NQ, DD], F8, name="a")
            wuv_sb = cpool.tile([P, NQ, WUVC], F8, name="wuv")
            betas_sb = cpool.tile([33, 2], F32, name="betas")
            dummy_sb = cpool.tile([P, S], BF16, name="dummy")

            # DMA order matches consumption order: wuv + the first rt
            # pair-slices feed the uv matmuls, a feeds proj wave 0.
            nc.scalar.dma_start(wuv_sb[:], wuv.ap())
            nc.scalar.dma_start(betas_sb[:], betas.ap())
            for q in range(NS):
                sl = slice(q * S, (q + 1) * S)
                if q == 1:  # spread the rt load over all three queues
                    nc.scalar.dma_start(rt_sb[:, 0:2, sl], rt.ap()[:, 0:2, sl])
                    nc.scalar.dma_start(rt_sb[:, 2:4, sl], rt.ap()[:, 2:4, sl])
                else:
                    nc.sync.dma_start(rt_sb[:, 0:2, sl], rt.ap()[:, 0:2, sl])
                    nc.gpsimd.dma_start(rt_sb[:, 2:4, sl], rt.ap()[:, 2:4, sl])
            nc.scalar.dma_start(a_sb[:], amat.ap())

            bt_sb = big.tile([P, NQ, NN], F8, name="bt")
            uvrow_sb = big.tile([33, NN], F8, name="uvrow")
            vcol_sb = big.tile([P, NM], F8, name="vcol")
            v_sb = big.tile([P, NM], F32, name="v")
            # su weights per m-chunk-pair: [pair, sub, col] with col 0 = u,
            # col 1 = ones, cols 2..15 zero padding (16B DoubleRow step).
            su_sb = big.tile([P, NPR, 2, SUC], F8, name="su")
            nc.vector.memset(su_sb[:], 0.0)
            nc.vector.memset(su_sb[:, :, :, 1:2], 1.0)

            # psR: 4 banks for the s/rowsum accumulators (also reused as
            # scratch for the uv projections before srs starts).
            # psMain: 4 banks as two rotating [128,1024] 2-bank tiles so the
            # exp ACTs can cover 1024 columns per instruction.
            psR = tc.alloc_tile_pool(name="psR", bufs=1, space="PSUM")
            psMain = tc.alloc_tile_pool(name="psMain", bufs=2, space="PSUM")
            srs = [
                psR.tile([P, S], F32, tag=f"srs{ns}", name=f"srs{ns}")
                for ns in range(NS)
            ]

            # ---- PE warmup: dummy matmuls (no input deps) un-throttle the
            # HAM clock gate while the input DMAs stream in.
            nc.vector.memset(dummy_sb[:], 0.0)
            for _ in range(5):
                t = psMain.tile([P, 2 * S], F32, tag="mm", name="mm")
                nc.tensor.matmul(
                    t[:, 0:S], dummy_sb[:, 0:P], dummy_sb[:], start=True, stop=True
                )

            # ---- uv: u (row 0) and v (row 32) from the [wtl|w2tl] lhsT;
            # one fused ACT per slice via per-partition scale/bias columns.
            for ns in range(NS):
                pur = srs[ns][0:WUVC, :]
                for pr in range(2):
                    nc.tensor.matmul(
                        pur[:],
                        wuv_sb[:, 2 * pr : 2 * pr + 2, :],
                        rt_sb[:, 2 * pr : 2 * pr + 2, ns * S : (ns + 1) * S],
                        start=(pr == 0),
                        stop=(pr == 1),
                        perf_mode=DR,
                        skip_group_check=True,
                    )
                nc.scalar.activation(
                    uvrow_sb[0:33, ns * S : (ns + 1) * S],
                    pur[0:33, :],
                    Ident,
                    bias=betas_sb[0:33, 1:2],
                    scale=betas_sb[0:33, 0:1],
                )

            # scatter rows [1, 2048] -> [128, pair, sub] columns off the PE:
            # bounce through flat DRAM, where the partition-scatter read
            # pattern is expressible.
            # v first: the v column gates the first exp (~24us in), while su
            # is not needed until the first srs matmul (~35us in). The two
            # 2048-descriptor scatters go on different queues so they don't
            # serialize.
            uv_dram = dpool.tile([2, NN], F8)
            nc.sync.dma_start(uv_dram[1:2, :], uvrow_sb[32:33, :])
            nc.sync.dma_start(uv_dram[0:1, :], uvrow_sb[0:1, :])
            with nc.allow_non_contiguous_dma(
                reason="2048-elem partition scatter, one-off"
            ):
                nc.sync.dma_start(
                    vcol_sb[:],
                    uv_dram[1, :].rearrange("(m p) -> p m", p=P),
                )
                nc.gpsimd.dma_start(
                    su_sb[:, :, :, 0],
                    uv_dram[0, :].rearrange("(pr i p) -> p pr i", p=P, i=2),
                )
            # gpsimd, not DVE: this copy waits on the scatter DMA and must
            # not block the DVE bt-cast chain behind it
            nc.gpsimd.tensor_copy(v_sb[:], vcol_sb[:])

            # ---- proj: B = A R^T, DoubleRow, cast to fp8 bt ----
            def proj(qo, half):
                t = psMain.tile([P, 2 * S], F32, tag="mm", name="mm")
                for sub in range(2):
                    ns = 2 * half + sub
                    po = t[:, sub * S : (sub + 1) * S]
                    for pr in range(2):
                        nc.tensor.matmul(
                            po[:],
                            a_sb[:, 2 * pr : 2 * pr + 2, qo * P : (qo + 1) * P],
                            rt_sb[:, 2 * pr : 2 * pr + 2, ns * S : (ns + 1) * S],
                            start=(pr == 0),
                            stop=(pr == 1),
                            perf_mode=DR,
                        )
                nc.vector.tensor_copy(
                    bt_sb[:, qo, half * 2 * S : (half + 1) * 2 * S], t[:]
                )

            # ---- gamma + exp ----
            ets = {}

            def gamma(mc):
                if mc % 2 == 0:
                    ets[mc // 2] = et_pool.tile(
                        [P, 2, NN], F8, tag="et", name="et"
                    )
                et = ets[mc // 2]
                for half in range(2):
                    g = psMain.tile([P, 2 * S], F32, tag="mm", name="mm")
                    for sub in range(2):
                        ns = 2 * half + sub
                        go = g[:, sub * S : (sub + 1) * S]
                        for pr in range(2):
                            nc.tensor.matmul(
                                go[:],
                                bt_sb[:, 2 * pr : 2 * pr + 2, mc * P : (mc + 1) * P],
                                rt_sb[:, 2 * pr : 2 * pr + 2, ns * S : (ns + 1) * S],
                                start=(pr == 0),
                                stop=(pr == 1),
                                perf_mode=DR,
                            )
                    nc.scalar.activation(
                        et[:, mc % 2, half * 2 * S : (half + 1) * 2 * S],
                        g[:],
                        Exp,
                        bias=v_sb[:, mc : mc + 1],
                        scale=SCALE / ASCALE,
                    )

            # ---- srs: s/rowsum accumulation over m chunk-pairs ----
            def srs_pair(pp):
                et = ets[pp]
                for ns in range(NS):
                    nc.tensor.matmul(
                        srs[ns][0:SUC, :],
                        su_sb[:, pp, :, :],
                        et[:, :, ns * S : (ns + 1) * S],
                        start=(pp == 0),
                        stop=(pp == NPR - 1),
                        perf_mode=DR,
                        skip_group_check=True,
                    )
                del ets[pp]

            # schedule: gamma chunks 0-7 run right after the first proj
            # n-half so the exp stream (the critical ACT work) starts early;
            # srs pairs trail their gamma chunks by >=1 full chunk.
            for qo in range(NQ):
                proj(qo, 0)
            proj(0, 1)
            for mc in range(0, 8):
                gamma(mc)
            for qo in range(1, NQ):
                proj(qo, 1)
            srs_pair(0)
            srs_pair(1)
            for mc in range(8, 12):
                gamma(mc)
            srs_pair(2)
            srs_pair(3)
            for mc in range(12, 16):
                gamma(mc)
            srs_pair(4)
            srs_pair(5)
            srs_pair(6)
            srs_pair(7)

            # copy the two result rows PSUM -> SBUF, then one DMA out.
            out_sb = big.tile([2, NN], F32, name="out_sb")
            for ns in range(NS):
                sl = slice(ns * S, (ns + 1) * S)
                # alternate DVE/ACT so the four drain copies run on two
                # engines in parallel
                if ns % 2 == 0:
                    nc.vector.tensor_copy(out_sb[:, sl], srs[ns][0:2, :])
                else:
                    nc.scalar.copy(out_sb[:, sl], srs[ns][0:2, :])
            nc.sync.dma_start(out.ap()[:], out_sb[:])
            psMain.release()
            psR.release()

    _split_multi_waits(nc)
    return nc


_NC = None


def _get_nc():
    global _NC
    if _NC is None:
        _NC = _build()
    return _NC


def kernel(R, Wq, bq, Wk, bk, Wv, bv, W1, b1, W2, b2):
    R = np.asarray(R, np.float32)
    Wq = np.asarray(Wq, np.float64)
    bq = np.asarray(bq, np.float64)
    Wk = np.asarray(Wk, np.float64)
    bk = np.asarray(bk, np.float64)
    Wv = np.asarray(Wv, np.float64)
    bv = np.asarray(bv, np.float64)
    W1 = np.asarray(W1, np.float64)
    b1 = np.asarray(b1, np.float64)
    W2 = np.asarray(W2, np.float64)
    b2 = np.asarray(b2, np.float64)

    # Collapse the linear head: winner = c.a + const, u = V c.
    c = W1.T @ W2[0]                      # [512]
    wtilde = Wv.T @ c                     # [512]
    beta = float(bv @ c)
    const = float(W2[0] @ b1 + b2[0])
    # Collapse the Q/K projections: gamma = R A R^T + v[m] (+ dropped n-term)
    at = Wk.T @ Wq                        # A^T = Wk^T Wq, [q', q]
    w2tilde = Wk.T @ bq                   # [512]
    beta2 = float(bq @ bk)

    # [p, chunk, d] with q = chunk*128 + p; x64 pre-scale for fp8 range
    a_h = np.ascontiguousarray(
        (at * ASCALE).reshape(NQ, P, DD).transpose(1, 0, 2)
    ).astype(F8NP)
    wuv_h = np.zeros((P, NQ, WUVC), F8NP)
    wuv_h[:, :, 0] = (wtilde * ASCALE).reshape(NQ, P).T.astype(F8NP)
    wuv_h[:, :, 32] = (w2tilde * ASCALE).reshape(NQ, P).T.astype(F8NP)
    # fused uv activation: per-partition (scale, bias) columns
    betas_h = np.zeros((33, 2), np.float32)
    betas_h[0] = (1.0 / ASCALE, beta)
    betas_h[32] = (SCALE / ASCALE, beta2 * SCALE)

    in_maps = []
    for b in range(NB):
        # [p, chunk, n]: R[b].T chunked over q so each q-chunk-pair slice
        # is a regular strided DMA
        rt_h = np.ascontiguousarray(
            R[b].T.reshape(NQ, P, NN).transpose(1, 0, 2)
        ).astype(F8NP)
        in_maps.append(
            {
                "rt": rt_h,
                "amat": a_h,
                "wuv": wuv_h,
                "betas": betas_h,
            }
        )

    nc = _get_nc()
    res = run_bass_kernel_spmd(nc, in_maps, core_ids=list(range(N_CORES)))
    outs = np.stack([res.results[b]["out"] for b in range(NB)])   # [8,2,2048]
    return (outs[:, 0] / outs[:, 1] + np.float32(const)).astype(np.float32)
